# revision 31
# baseline (speedup 1.0000x reference)
"""Trainium2 Bass kernel for nn_Classifier (attention-pool + linear + classifier).

Reference math (per state n of 64):
    attn  = softmax(output_set @ states[n].T, axis=-1)      # [64io, 512s]
    mix   = attn @ states[n]                                # [64io, 1024h]
    o     = [mix | output_set] @ Wo + bo                    # [64io, 1024h]
    logit = tanh(o).flatten() @ Wc + bc                     # [64]

Sharding: data-parallel over the leading n_states dim - 8 states per core on
8 cores. Each core computes its own [8, 64] logits slice; host concatenates.

Per-core schedule (v3):
  - every DRAM tensor is host-pre-shuffled into its SBUF layout
    ([partition, ...contiguous free dims]) so each DMA generates 128
    large contiguous descriptors instead of thousands of 1KB ones
    (the v2 descriptor-ring stalls came from rearranged DMA APs).
  - the attention operands (statesT, output_set) travel as fp8e4m3
    scaled by 64: halves their HBM bytes; the softmax makes the attn
    weights insensitive to fp8 score noise (verified 3.18e-3 vs
    3.15e-3 all-bf16 in numpy). The x64^2 score scale is compensated
    exactly by the Exp activation's scale parameter.
  - softmax skips max-subtraction (scores bounded ~|0.4|), and the
    1/Z normalize is folded into the mix PSUM evacuation.
  - states stream on the sync DMA queue, weights on the scalar queue;
    wc chunks are issued inside the pair loop behind Exp ops so their
    descriptor generation is naturally paced after the states.
  - PE program order is software-pipelined across state pairs.
"""

import os
import sys

import numpy as np

for _p in ("/opt/trn_rl_repo",):
    if _p not in sys.path:
        sys.path.insert(0, _p)

import concourse.bass as bass
import concourse.mybir as mybir
import concourse.tile as tile
from concourse import bacc
from concourse.masks import make_identity

IO, H, S, NTOT = 64, 1024, 512, 64
NCORES = 8
NLOC = NTOT // NCORES  # states per core
P = 128
HC = H // P  # 8 h-chunks
SC = S // P  # 4 s-chunks
NPAIR = NLOC // 2

import ml_dtypes

DT = mybir.dt.bfloat16
NPDT = ml_dtypes.bfloat16

USE_FP8 = os.environ.get("KBASS_FP8", "1") != "0"
F8 = mybir.dt.float8e4
NPF8 = getattr(ml_dtypes, "float8_e4m3fn", None) or ml_dtypes.float8_e4m3
F8_SCALE = 64.0
if not USE_FP8:
    F8 = DT
    NPF8 = NPDT
    F8_SCALE = 1.0

F32 = mybir.dt.float32
AX = mybir.AxisListType
AF = mybir.ActivationFunctionType

ST_BUFS = 6  # states (of each orientation) in flight
NJG = 8  # i-pairs per wc DMA chunk


def build_bass(reps=1):
    nc = bacc.Bacc(
        "TRN2", target_bir_lowering=False, debug=False, num_devices=NCORES
    )

    # all DRAM tensors are pre-shuffled host-side into [partition, free...]
    stT_d = nc.declare_dram_parameter("stT", [NLOC, P, HC, S], F8, isOutput=False)
    sn_d = nc.declare_dram_parameter("sn", [NLOC, P, SC, H], F8, isOutput=False)
    osT2f8_d = nc.declare_dram_parameter("osT2f8", [P, HC, 2 * IO], F8, isOutput=False)
    osT2_d = nc.declare_dram_parameter("osT2", [P, HC, 2 * IO], DT, isOutput=False)
    wo_top_d = nc.declare_dram_parameter("wo_top", [P, HC, H], DT, isOutput=False)
    wo_bot_d = nc.declare_dram_parameter("wo_bot", [P, HC, H], DT, isOutput=False)
    bo2_d = nc.declare_dram_parameter("bo2", [P, H], DT, isOutput=False)
    # classifier weights, pair-packed: [hp, j, hc, t*64+c] = Wc[(2j+t)*H + hc*128 + hp, c]
    wc_d = nc.declare_dram_parameter("wc", [P, IO // 2, HC, P], DT, isOutput=False)
    bct_d = nc.declare_dram_parameter("bct", [IO, NLOC], F32, isOutput=False)
    foldF_d = nc.declare_dram_parameter("foldF", [P, IO], DT, isOutput=False)
    out_d = nc.declare_dram_parameter("logitsT", [IO, NLOC], F32, isOutput=True)

    with tile.TileContext(nc) as tc:
        with (
            tc.tile_pool(name="consts", bufs=1) as consts,
            tc.tile_pool(name="stT", bufs=ST_BUFS) as stT_pool,
            tc.tile_pool(name="sn", bufs=ST_BUFS) as sn_pool,
            tc.tile_pool(name="work", bufs=2) as work,
            tc.tile_pool(name="sm", bufs=4) as sm_pool,
            tc.tile_pool(name="ps_attn", bufs=2, space="PSUM") as ps_attn,
            tc.tile_pool(name="ps_tr", bufs=2, space="PSUM") as ps_tr,
            tc.tile_pool(name="ps_mix", bufs=1, space="PSUM") as ps_mix,
            tc.tile_pool(name="ps_o", bufs=1, space="PSUM") as ps_o,
        ):
            # ---- SBUF-resident tensors ----
            osT2f8_sb = consts.tile([P, HC, 2 * IO], F8)
            osT2_sb = consts.tile([P, HC, 2 * IO], DT)
            wo_top_sb = consts.tile([P, HC, H], DT)
            wo_bot_sb = consts.tile([P, HC, H], DT)
            ident = consts.tile([P, P], DT)
            bo2_sb = consts.tile([P, H], DT)
            bct_sb = consts.tile([IO, NLOC], F32)
            foldF_sb = consts.tile([P, IO], DT)
            const_sb = consts.tile([P, H], DT)
            wc_sb = consts.tile([P, IO // 2, HC, P], DT)
            # tanh(o) transposed, state-major: [hp, hc, state, io]
            tT_all = consts.tile([P, HC, NLOC, IO], DT)

            stT = {}
            sn = {}

            def fetch_pair(pi):
                # stT of both states first: the col-tiled attn MMs need both,
                # and they're small (fp8); sn (bf16) follows
                for st in (2 * pi, 2 * pi + 1):
                    stT[st] = stT_pool.tile(
                        [P, HC, S], F8, tag="stT", name=f"stT_{st}"
                    )
                    nc.sync.dma_start(stT[st][:], stT_d[st])
                for st in (2 * pi, 2 * pi + 1):
                    sn[st] = sn_pool.tile([P, SC, H], F8, tag="sn", name=f"sn_{st}")
                    nc.sync.dma_start(sn[st][:], sn_d[st])

            # states stream on the sync queue; weights on the scalar queue.
            # wc rides the END of the sync queue: ring order guarantees its
            # transfers never race the states for HBM bandwidth.
            nc.sync.dma_start(osT2f8_sb[:], osT2f8_d[:])
            fetch_pair(0)
            nc.scalar.dma_start(osT2_sb[:], osT2_d[:])
            nc.scalar.dma_start(wo_bot_sb[:], wo_bot_d[:])
            nc.scalar.dma_start(bo2_sb[:], bo2_d[:])
            nc.scalar.dma_start(bct_sb[:], bct_d[:])
            nc.scalar.dma_start(foldF_sb[:], foldF_d[:])
            fetch_pair(1)
            nc.scalar.dma_start(wo_top_sb[:], wo_top_d[:])
            make_identity(nc, ident[:])
            fetch_pair(2)

            def fetch_wc(jg):
                nc.sync.dma_start(
                    wc_sb[:, jg * NJG : (jg + 1) * NJG],
                    wc_d[:, jg * NJG : (jg + 1) * NJG],
                )

            fetch_wc(0)
            fetch_pair(3)
            fetch_wc(1)
            fetch_wc(2)
            fetch_wc(3)

            for _rep in range(reps):

                def attn_mms(pi):
                    a, b = 2 * pi, 2 * pi + 1
                    aps = ps_attn.tile([P, S], F32, tag="ps_attn", name=f"aps_{pi}")
                    for hc in range(HC):
                        for s_i, st in ((0, a), (1, b)):
                            nc.tensor.matmul(
                                aps[s_i * IO : (s_i + 1) * IO, :],
                                lhsT=osT2f8_sb[:, hc, s_i * IO : (s_i + 1) * IO],
                                rhs=stT[st][:, hc, :],
                                start=(hc == 0),
                                stop=(hc == HC - 1),
                                tile_position=(0, s_i * IO),
                                skip_group_check=True,
                            )
                    return aps

                def tanh_transposes(pi, t_sb):
                    # t^T into the classifier operand buffer (state-major cols
                    # make the evacuation copies contiguous; copies on DVE so
                    # the ACT queue stays free for Exp/Tanh)
                    ttps = [
                        ps_tr.tile([P, 512], DT, tag="ps_tr", name=f"ttps_{pi}_{j}")
                        for j in range(2)
                    ]
                    for hc in range(HC):
                        nc.tensor.transpose(
                            ttps[hc // 4][:, (hc % 4) * P : (hc % 4 + 1) * P],
                            t_sb[:, hc * P : (hc + 1) * P],
                            ident[:],
                        )
                    for hc in range(HC):
                        src = ttps[hc // 4][:, (hc % 4) * P : (hc % 4 + 1) * P]
                        nc.vector.tensor_copy(
                            tT_all[:, hc, 2 * pi : 2 * pi + 2, :],
                            src.rearrange("p (n io) -> p n io", n=2),
                        )

                def softmax_pieces(pi, aps):
                    # no max-subtraction: scores are bounded ~|0.4| by the 0.05
                    # input scaling; the fp8 x64 pre-scales cancel via `scale`
                    exps = work.tile([P, S], DT, tag="exps", name=f"exps_{pi}")
                    sumexp = sm_pool.tile([P, 1], F32, tag="sumexp", name=f"se_{pi}")
                    nc.scalar.activation(
                        exps[:],
                        aps[:],
                        AF.Exp,
                        scale=1.0 / (F8_SCALE * F8_SCALE),
                        accum_out=sumexp[:],
                    )
                    rinv = sm_pool.tile([P, 1], F32, tag="rinv", name=f"ri_{pi}")
                    nc.vector.reciprocal(rinv[:], sumexp[:])
                    return exps, rinv

                # ---- software-pipelined pair loop ----
                # bootstrap: attn(0) first (needs only osT2f8+stT0), const MMs
                # fill the Exp(0) latency
                aps_t = {0: attn_mms(0)}
                cps = ps_o.tile([P, H], F32, tag="ps_o", name="cps")
                for hc in range(HC):
                    for hh in range(2):
                        nc.tensor.matmul(
                            cps[:, hh * 512 : (hh + 1) * 512],
                            lhsT=osT2_sb[:, hc, :],
                            rhs=wo_bot_sb[:, hc, hh * 512 : (hh + 1) * 512],
                            start=(hc == 0),
                            stop=(hc == HC - 1),
                        )
                sm_t = {0: softmax_pieces(0, aps_t[0])}
                aps_t[1] = attn_mms(1)
                sm_t[1] = softmax_pieces(1, aps_t[1])
                t_sb_t = {}
                for pi in range(NPAIR):
                    a, b = 2 * pi, 2 * pi + 1

                    exps, rinv = sm_t.pop(pi)
                    aps_t.pop(pi)

                    # attn^T via PE transposes, chunk-chained into mix MMs;
                    # the evacuation copies cast to fp8 (the mix matmul runs
                    # fp8xfp8: its x64-scaled sn operand is compensated by
                    # the host-side Wo_top/64)
                    atps = ps_tr.tile([P, 512], DT, tag="ps_tr", name=f"atps_{pi}")
                    attnT = work.tile([P, SC, P], F8, tag="attnT", name=f"attnT_{pi}")
                    for sc in range(SC):
                        nc.tensor.transpose(
                            atps[:, sc * P : (sc + 1) * P],
                            exps[:, sc * P : (sc + 1) * P],
                            ident[:],
                        )
                    for sc in range(SC):
                        nc.vector.tensor_copy(
                            attnT[:, sc, :], atps[:, sc * P : (sc + 1) * P]
                        )

                    # pair pi-1 tanh transposes here: their DVE evacuations
                    # finish during the mix MMs, so the mix-evac multiply
                    # isn't stuck behind them in the DVE queue
                    if pi > 0:
                        tanh_transposes(pi - 1, t_sb_t.pop(pi - 1))

                    # attn runs two pairs ahead (2 PSUM bufs): Exp(pi+1) is
                    # already done, and Exp(pi+2) completes during S4-S7, so
                    # each iteration's transposes start with no ACT wait
                    if pi + 2 < NPAIR:
                        aps_t[pi + 2] = attn_mms(pi + 2)
                        sm_t[pi + 2] = softmax_pieces(pi + 2, aps_t[pi + 2])

                    # mix (unnormalized) = exps @ states
                    mps = ps_mix.tile([P, H], F32, tag="ps_mix", name=f"mps_{pi}")
                    for sc in range(SC):
                        for s_i, st in ((0, a), (1, b)):
                            for hh in range(2):
                                nc.tensor.matmul(
                                    mps[
                                        s_i * IO : (s_i + 1) * IO,
                                        hh * 512 : (hh + 1) * 512,
                                    ],
                                    lhsT=attnT[:, sc, s_i * IO : (s_i + 1) * IO],
                                    rhs=sn[st][:, sc, hh * 512 : (hh + 1) * 512],
                                    start=(sc == 0),
                                    stop=(sc == SC - 1),
                                    tile_position=(0, s_i * IO),
                                    skip_group_check=True,
                                )

                    if pi == 0:
                        nc.vector.tensor_add(const_sb[:], cps[:], bo2_sb[:])

                    # evacuate mix with the softmax normalize folded in
                    mix_sb = work.tile([P, H], DT, tag="mix_sb", name=f"mix_{pi}")
                    nc.vector.tensor_scalar_mul(mix_sb[:], mps[:], rinv[:])

                    # mix^T, chunk-chained into o MMs
                    mtps = [
                        ps_tr.tile([P, 512], DT, tag="ps_tr", name=f"mtps_{pi}_{j}")
                        for j in range(2)
                    ]
                    mixT = work.tile([P, HC, P], DT, tag="mixT", name=f"mixT_{pi}")
                    for hc in range(HC):
                        nc.tensor.transpose(
                            mtps[hc // 4][:, (hc % 4) * P : (hc % 4 + 1) * P],
                            mix_sb[:, hc * P : (hc + 1) * P],
                            ident[:],
                        )
                    for hc in range(HC):
                        nc.vector.tensor_copy(
                            mixT[:, hc, :],
                            mtps[hc // 4][:, (hc % 4) * P : (hc % 4 + 1) * P],
                        )

                    # o = mix @ Wo_top + const, by halves: the const lands in
                    # PSUM via an identity-matmul accumulate (start=False) so
                    # Tanh reads PSUM directly - no DVE add on the chain
                    ops_ = ps_o.tile([P, H], F32, tag="ps_o", name=f"ops_{pi}")
                    t_sb = work.tile([P, H], DT, tag="t_sb", name=f"t_{pi}")
                    for hh in range(2):
                        cols = slice(hh * 512, (hh + 1) * 512)
                        for hc in range(HC):
                            for s_i in (0, 1):
                                nc.tensor.matmul(
                                    ops_[s_i * IO : (s_i + 1) * IO, cols],
                                    lhsT=mixT[:, hc, s_i * IO : (s_i + 1) * IO],
                                    rhs=wo_top_sb[:, hc, cols],
                                    start=(hc == 0),
                                    stop=False,
                                    tile_position=(0, s_i * IO),
                                    skip_group_check=True,
                                )
                        nc.tensor.matmul(
                            ops_[:, cols],
                            lhsT=ident[:],
                            rhs=const_sb[:, cols],
                            start=False,
                            stop=True,
                            skip_group_check=True,
                        )
                        nc.scalar.activation(
                            t_sb[:, cols], ops_[:, cols], AF.Tanh
                        )
                    t_sb_t[pi] = t_sb

                tanh_transposes(NPAIR - 1, t_sb_t.pop(NPAIR - 1))

                # ---- classifier, i-pair packed (valid quadrants disjoint):
                # lhsT = [Wc_{2j} | Wc_{2j+1}] (128 cols), rhs = tT (state, parity)
                # psum rows 0:64 even-i partials at even cols, 64:128 odd at odd.
                lgps = ps_attn.tile([P, 2 * NLOC], F32, tag="ps_attn", name="lgps")
                for j in range(IO // 2):
                    for hc in range(HC):
                        nc.tensor.matmul(
                            lgps[:],
                            lhsT=wc_sb[:, j, hc, :],
                            rhs=tT_all[:, hc, :, 2 * j : 2 * j + 2],
                            start=(j == 0 and hc == 0),
                            stop=(j == IO // 2 - 1 and hc == HC - 1),
                            skip_group_check=True,
                        )
                # epilogue: logitsT = q_even + q_odd + bc. The cross-partition
                # fold (rows 64:128 onto 0:64) happens via a tiny matmul with
                # the [I;I] fold matrix instead of a slow SWDGE accum-DMA.
                lg_v = lgps[:].rearrange("p (n u) -> p n u", u=2)
                lt2_sb = work.tile([P, NLOC], DT, tag="lt2_sb")
                nc.vector.tensor_copy(lt2_sb[0:IO, :], lg_v[0:IO, :, 0])
                nc.vector.tensor_copy(lt2_sb[IO:P, :], lg_v[IO:P, :, 1])
                ltps = ps_tr.tile([IO, NLOC], F32, tag="ps_tr", name="ltps")
                nc.tensor.matmul(
                    ltps[:], lhsT=foldF_sb[:], rhs=lt2_sb[:], start=True, stop=True
                )
                lt_sb = work.tile([IO, NLOC], F32, tag="lt_sb")
                nc.vector.tensor_add(lt_sb[:], ltps[:], bct_sb[:])
                nc.scalar.dma_start(out_d[:], lt_sb[:])

    nc.compile()
    return nc


def make_in_maps(states, output_set, Wo, bo, Wc, bc):
    """Build the per-core input maps (host-side sharding + layout prep).

    Every tensor is shuffled into its on-chip SBUF layout [partition, free...]
    so DMAs are contiguous per partition.
    """
    states = np.asarray(states, dtype=np.float32)
    output_set = np.asarray(output_set, dtype=np.float32)
    Wo = np.asarray(Wo, dtype=np.float32)
    bo = np.asarray(bo, dtype=np.float32)
    Wc = np.asarray(Wc, dtype=np.float32)
    bc = np.asarray(bc, dtype=np.float32)

    def p_shuffle(x):  # [(c p), f...] -> [p, c, f...]
        return np.ascontiguousarray(
            x.reshape((-1, P) + x.shape[1:]).swapaxes(0, 1)
        )

    osT = output_set.T  # [H, IO]
    osT2 = np.concatenate([osT, osT], axis=1)  # [H, 2IO]
    shared = {
        "osT2f8": p_shuffle(osT2 * F8_SCALE).astype(NPF8),
        "osT2": p_shuffle(osT2).astype(NPDT),
        # mix arrives x64 (fp8-scaled sn); compensate in Wo_top, which is
        # exponent-limited only (bf16) so /64 is lossless
        "wo_top": p_shuffle(Wo[:H] / F8_SCALE).astype(NPDT),
        "wo_bot": p_shuffle(Wo[H:]).astype(NPDT),
        "bo2": np.ascontiguousarray(np.tile(bo, (P, 1))).astype(NPDT),
        # Wc[(2j+t)*H + hc*128 + hp, c] -> [hp, j, hc, t*64+c]
        "wc": np.ascontiguousarray(
            Wc.reshape(IO // 2, 2, HC, P, IO)
            .transpose(3, 0, 2, 1, 4)
            .reshape(P, IO // 2, HC, P)
        ).astype(NPDT),
        "bct": np.ascontiguousarray(np.tile(bc[:, None], (1, NLOC))).astype(
            np.float32
        ),
        "foldF": np.ascontiguousarray(
            np.concatenate([np.eye(IO), np.eye(IO)], axis=0)
        ).astype(NPDT),
    }
    in_maps = []
    for k in range(NCORES):
        sl = states[k * NLOC : (k + 1) * NLOC]  # [NLOC, S, H]
        stT_full = sl.transpose(0, 2, 1) * F8_SCALE  # [NLOC, H, S]
        in_maps.append(
            {
                # [NLOC, p, hc, s] and [NLOC, p, sc, h]
                "stT": np.ascontiguousarray(
                    stT_full.reshape(NLOC, HC, P, S).swapaxes(1, 2)
                ).astype(NPF8),
                "sn": np.ascontiguousarray(
                    (sl * F8_SCALE).reshape(NLOC, SC, P, H).swapaxes(1, 2)
                ).astype(NPF8),
                **shared,
            }
        )
    return in_maps


_NC_CACHE = {}


def get_nc(reps=1):
    if reps not in _NC_CACHE:
        _NC_CACHE[reps] = build_bass(reps)
    return _NC_CACHE[reps]


def kernel(states, output_set, Wo, bo, Wc, bc):
    from concourse.bass_utils import run_bass_kernel_spmd

    nc = get_nc()
    in_maps = make_in_maps(states, output_set, Wo, bo, Wc, bc)
    res = run_bass_kernel_spmd(nc, in_maps, core_ids=list(range(NCORES)))
    out = np.concatenate(
        [np.asarray(res.results[k]["logitsT"]).T for k in range(NCORES)], axis=0
    )
    return out.astype(np.float32)
